# revision 24
# baseline (speedup 1.0000x reference)
"""Trainium2 Bass kernel for CustomMHA (B=4, L=2048, D=1024, H=16, DK=64), fp32.

Sharding: 8 cores = 4 batches x 2 head-groups (8 heads each).
Each core computes, for its (batch b, head-group g):
  qkv = x_b @ Win_slice.T + b_slice       (f16 matmuls, fp32 accum)
  per head: S^T = k q^T; A = exp(S^T*scale); z^T = [v|1|0pad]^T A
  partial_out = z_hat @ proj_slice.T      ([2048, 1024] fp32, no proj_b)
Host sums the two head-group partials per batch and adds proj_b.

v3 notes: the z weights are [v_h | ones | zero-pad] padded to the full 128
columns — HW-measured, full-width (128-col) weight loads get FWL + the
background weight buffer and back-to-back matmuls pipeline at ~216ns,
while partial-width loads (65/64/1 col) pay ~+120ns of array-passage
serialization per weight switch.  The ones column yields the softmax
denominator in psum row 64 for free; rows 65-127 are garbage and never
read.  Emission is software-pipelined per (qc, pr) unit: each 2-kt block
emits z of the previous unit, a filler chunk (qkv/v/proj), then S+exp of
the current unit, keeping ACT (the exp engine, ~270us busy) fed while the
PE (~275us of streams) hides under it.  zt is stored as per-qc tiles so
proj reads (prev qc) never serialize against normalize writes (cur qc).

Layout notes (per core):
  xT    [1024, 2048] f16  (x_b transposed; k-tiles are matmul lhsT/rhs)
  wqkT  [1024, 1024] f16  (rows [Wq_g; Wk_g] transposed)
  wvT   [1024, 512]  f16
  bqk   [128, 8] f32      (bias for q,k features; [partition, feature-tile])
  bv    [1, 512] f16
  projT [512, 1024] f16   (proj_w[:, g-cols] transposed)
  out   [2048, 1024] f32

The kernel is self-contained: shapes/sharding hardcoded, no file reads.
"""

import numpy as np
from contextlib import ExitStack

import concourse.bass as bass
import concourse.mybir as mybir
import concourse.tile as tile
from concourse import bacc
from concourse.bass_utils import run_bass_kernel_spmd

# Problem constants
B, L, D, H = 4, 2048, 1024, 16
DK = D // H                     # 64
SCALE = 1.0 / float(np.sqrt(DK))

# Per-core constants
P = 128
T = L                           # tokens per core (one batch)
NH = H // 2                     # 8 heads per core
DH = NH * DK                    # 512
NKT = D // P                    # 8 k-tiles over model dim
NTT = T // P                    # 16 token tiles
NQC = T // 512                  # 4 query chunks of 512
NU = NQC * (NH // 2)            # 16 (qc, pr) units
F16 = mybir.dt.float16
F32 = mybir.dt.float32

# PSUM budget (8 banks): tag "s" (S^T pair tiles, 2 banks each) x2 bufs = 4,
# tag "z" (z accumulators) x2 = 2, tag "m" (qkv/v/proj) x2 = 2.


def build_program(reps=1):
    nc = bacc.Bacc("TRN2", target_bir_lowering=False, debug=False,
                   enable_asserts=False, num_devices=8)

    xT = nc.dram_tensor("xT", [D, T], F16, kind="ExternalInput").ap()
    wqkT = nc.dram_tensor("wqkT", [D, 2 * DH], F16, kind="ExternalInput").ap()
    wvT = nc.dram_tensor("wvT", [D, DH], F16, kind="ExternalInput").ap()
    bqk = nc.dram_tensor("bqk", [P, NKT], F32, kind="ExternalInput").ap()
    bv = nc.dram_tensor("bv", [1, DH], F16, kind="ExternalInput").ap()
    projT = nc.dram_tensor("projT", [DH, D], F16, kind="ExternalInput").ap()
    out = nc.dram_tensor("out", [T, D], F32, kind="ExternalOutput").ap()

    with tile.TileContext(nc) as tc:
        with ExitStack() as ctx:
            _emit(nc, tc, ctx, xT, wqkT, wvT, bqk, bv, projT, out, reps)
    nc.compile()
    return nc


def _emit(nc, tc, ctx, xT, wqkT, wvT, bqk, bv, projT, out, reps=1):
    pers = ctx.enter_context(tc.tile_pool(name="pers", bufs=1))
    apool = ctx.enter_context(tc.tile_pool(name="apool", bufs=19))
    rpool = ctx.enter_context(tc.tile_pool(name="rpool", bufs=3))
    opool = ctx.enter_context(tc.tile_pool(name="opool", bufs=2))
    pspool = ctx.enter_context(tc.tile_pool(name="pspool", bufs=1, space="PSUM"))

    # ---- constant / weight / input loads ----
    bqk_sb = pers.tile([P, NKT], F32, name="bqk_sb")
    nc.sync.dma_start(bqk_sb[:], bqk[:])
    bv_sb = pers.tile([1, DH], F16, name="bv_sb")
    nc.sync.dma_start(bv_sb[:], bv[:])
    bvB = pers.tile([P, DH], F16, name="bvB")
    nc.gpsimd.partition_broadcast(bvB[:], bv_sb[:])
    # dummy exp: pulls the one-time ACT exp-table load (~2.7us) into the DMA
    # head so the first real S-tile exp doesn't pay it
    warm = pers.tile([P, NKT], F16, name="warm")
    nc.scalar.activation(warm[:], bqk_sb[:],
                         mybir.ActivationFunctionType.Exp, scale=0.001)

    wqk_sb = []
    x_sb = []
    wv_sb = []
    for ki in range(NKT):
        w = pers.tile([P, 2 * DH], F16, name=f"wqk_sb{ki}")
        nc.sync.dma_start(w[:], wqkT[ki * P:(ki + 1) * P, :])
        wqk_sb.append(w)
        xx = pers.tile([P, T], F16, name=f"x_sb{ki}")
        nc.sync.dma_start(xx[:], xT[ki * P:(ki + 1) * P, :])
        x_sb.append(xx)
    for ki in range(NKT):
        w = pers.tile([P, DH], F16, name=f"wv_sb{ki}")
        nc.sync.dma_start(w[:], wvT[ki * P:(ki + 1) * P, :])
        wv_sb.append(w)
    projT_sb = []
    for ki in range(DH // P):
        w = pers.tile([P, D], F16, name=f"projT_sb{ki}")
        nc.sync.dma_start(w[:], projT[ki * P:(ki + 1) * P, :])
        projT_sb.append(w)

    qk_sb = [pers.tile([P, T], F16, name=f"qk_sb{mi}") for mi in range(NKT)]
    # z weights, full 128 columns per head: [v (64) | ones (1) | zeros (63)]
    vbuf = [pers.tile([P, NH, P], F16, name=f"vbuf{ti}") for ti in range(NTT)]
    # zt as per-qc tiles: proj (prev qc) and normalize (cur qc) stay disjoint
    zt_sb = [[pers.tile([P, 512], F16, name=f"zt_sb{ki}_{qc}")
              for qc in range(NQC)] for ki in range(DH // P)]

    for ti in range(NTT):
        nc.vector.memset(vbuf[ti][:, :, DK:], 0.0)
        nc.vector.memset(vbuf[ti][:, :, DK:DK + 1], 1.0)

    _emit_compute(nc, tc, pers, apool, rpool, opool, pspool,
                  wqk_sb, x_sb, wv_sb, projT_sb, qk_sb, vbuf, zt_sb,
                  bqk_sb, bvB, out, reps)


def _emit_compute(nc, tc, pers, apool, rpool, opool, pspool,
                  wqk_sb, x_sb, wv_sb, projT_sb, qk_sb, vbuf, zt_sb,
                  bqk_sb, bvB, out, reps=1):

    def emit_qk(mi, tcn):
        ps = pspool.tile([P, 512], F32, tag="m", bufs=2, name=f"ps_qk{mi}_{tcn}")
        for ki in range(NKT):
            nc.tensor.matmul(
                ps[:],
                wqk_sb[ki][:, mi * P:(mi + 1) * P],
                x_sb[ki][:, tcn * 512:(tcn + 1) * 512],
                start=(ki == 0), stop=(ki == NKT - 1),
            )
        nc.vector.tensor_scalar_add(
            qk_sb[mi][:, tcn * 512:(tcn + 1) * 512], ps[:],
            bqk_sb[:, mi:mi + 1],
        )

    def emit_v(ti):
        ps = pspool.tile([P, 512], F32, tag="m", bufs=2, name=f"ps_v{ti}")
        for ki in range(NKT):
            nc.tensor.matmul(
                ps[:],
                x_sb[ki][:, ti * P:(ti + 1) * P],
                wv_sb[ki][:],
                start=(ki == 0), stop=(ki == NKT - 1),
            )
        nc.vector.tensor_add(
            vbuf[ti][:, :, 0:DK],
            ps.rearrange("p (h j) -> p h j", h=NH),
            bvB.rearrange("p (h j) -> p h j", h=NH),
        )

    def emit_proj(ti, ocn):
        qc, tsub = divmod(ti, NQC)
        ps = pspool.tile([P, 512], F32, tag="m", bufs=2, name=f"ps_o{ti}_{ocn}")
        for ki in range(DH // P):
            nc.tensor.matmul(
                ps[:],
                zt_sb[ki][qc][:, tsub * P:(tsub + 1) * P],
                projT_sb[ki][:, ocn * 512:(ocn + 1) * 512],
                start=(ki == 0), stop=(ki == DH // P - 1),
            )
        ot = opool.tile([P, 512], F32, tag="ot", name=f"ot{ti}_{ocn}")
        nc.vector.tensor_copy(ot[:], ps[:])
        nc.sync.dma_start(out[ti * P:(ti + 1) * P, ocn * 512:(ocn + 1) * 512],
                          ot[:])

    # Head pairs are row-packed on the PE for S: even head uses array rows
    # 0-63, odd head rows 64-127; the two S matmuls run concurrently.  One
    # psum tile [128, 2, 512] holds both heads' S^T chunk for a kt tile;
    # one exp instruction covers both.
    def emit_S(qc, pr, kt):
        qcs = slice(qc * 512, (qc + 1) * 512)
        qtile = qk_sb[pr]
        ktile = qk_sb[4 + pr]
        kts = slice(kt * P, (kt + 1) * P)
        ps = pspool.tile([P, 2, 512], F32, tag="s", bufs=2,
                         name=f"ps_s{qc}_{pr}_{kt}")
        nc.tensor.matmul(ps[:, 0, :], ktile[0:64, kts], qtile[0:64, qcs],
                         start=True, stop=True, tile_position=(0, 0))
        nc.tensor.matmul(ps[:, 1, :], ktile[64:128, kts], qtile[64:128, qcs],
                         start=True, stop=True, tile_position=(64, 0))
        a = apool.tile([P, 2, 512], F16, tag="A", name=f"a_{qc}_{pr}_{kt}")
        nc.scalar.activation(a, ps[:], mybir.ActivationFunctionType.Exp,
                             scale=SCALE)
        return a

    # z for one head: full-width padded weights, psum rows 0-63 = z, row 64 =
    # denominator, rows 65-127 garbage.  Two psum accumulators (head e, o)
    # per unit.
    def emit_z_kt(zps_e, zps_o, pr, a_tiles, kt):
        lhe, lho = 2 * pr, 2 * pr + 1
        nc.tensor.matmul(
            zps_e[:], vbuf[kt][:, lhe, :], a_tiles[kt][:, 0, :],
            start=(kt == 0), stop=(kt == NTT - 1),
        )
        nc.tensor.matmul(
            zps_o[:], vbuf[kt][:, lho, :], a_tiles[kt][:, 1, :],
            start=(kt == 0), stop=(kt == NTT - 1),
        )

    def emit_normalize(qc, pr, zps_e, zps_o):
        for idx, zps in ((0, zps_e), (1, zps_o)):
            row = 64 * idx
            # reciprocal_approx_fast only honors base-partition-0 inputs,
            # so stage the denominator row into partition 0 first; the
            # approx (~18 bits) is ~5x faster than exact reciprocal.
            den = rpool.tile([1, 512], F32, tag="dn", name=f"dn_{qc}_{pr}_{idx}")
            nc.vector.tensor_copy(den[:], zps[DK:DK + 1, :])
            recip = rpool.tile([1, 512], F32, tag="rc", name=f"rc_{qc}_{pr}_{idx}")
            nc.vector.reciprocal_approx_fast(recip[:], den[:])
            recipB = rpool.tile([64, 512], F32, tag="rb", name=f"rb_{qc}_{pr}_{idx}")
            nc.gpsimd.partition_broadcast(recipB[:], recip[:])
            nc.vector.tensor_mul(
                zt_sb[pr][qc][row:row + 64, :],
                zps[0:DK, :],
                recipB[:],
            )

    # ---- flat cross-rep software-pipelined schedule ----
    # Units U = 0..16*reps-1; rel u = U%16 maps to (qc, pr), qc-major.
    # Phase U emits S+exp of unit U interleaved with z of unit U-1 and
    # filler chunks (qkv/v/proj); normalize(U-1) closes the phase.  The
    # pipeline runs straight across rep boundaries: rep r+1's qk tiles are
    # queued during rep r's last phases (after rep r's final S reads of
    # each tile), its v tiles pop just behind rep r's last-unit z reads,
    # and rep r's qc3 proj chunks pop during rep r+1's early phases.
    units = [(qc, pr) for qc in range(NQC) for pr in range(NH // 2)]

    fillers = []

    def fill_qk(mi, tcn):
        fillers.append(lambda mi=mi, tcn=tcn: emit_qk(mi, tcn))

    def fill_v(ti):
        fillers.append(lambda ti=ti: emit_v(ti))

    def fill_proj(ti, ocn):
        fillers.append(lambda ti=ti, ocn=ocn: emit_proj(ti, ocn))

    def pop_fill(n=1):
        for _ in range(n):
            if fillers:
                fillers.pop(0)()

    # Head (rep 0 only): k0 (all chunks) + q0 chunk 0, emitted directly.
    for tcn in range(NQC):
        emit_qk(4, tcn)
    emit_qk(0, 0)

    a_prev = None
    NUF = NU * reps
    for U in range(NUF):
        r, u = divmod(U, NU)
        qc, pr = units[u]
        # === filler queueing (a chunk must pop no later than the block of
        # its first consumer; pops are 2/block in phases with v mass,
        # 1/block otherwise) ===
        if U == 0:
            fill_qk(1, 0)
            for tcn in range(NQC):
                fill_qk(4 + 1, tcn)
            for ti in range(11):
                fill_v(ti)
        elif r == 0 and u == 1:
            for ti in range(11, NTT):
                fill_v(ti)
            fill_qk(2, 0)
            for tcn in range(NQC):
                fill_qk(4 + 2, tcn)
        elif r > 0 and u == 0:
            # rep r's v tiles: vbuf[kt] write waits rep r-1's last z read
            # (same block, just ahead); consumed by z(r, u0) next phase
            for ti in range(NTT):
                fill_v(ti)
        elif r > 0 and u == 1:
            # previous rep's qc3 proj (zt[.][3] final since end of prev phase)
            for ti in range(4 * (NQC - 1), 4 * NQC):
                for ocn in range(2):
                    fill_proj(ti, ocn)
            fill_qk(2, 0)
            for tcn in range(NQC):
                fill_qk(4 + 2, tcn)
        elif u == 2:
            fill_qk(3, 0)
            for tcn in range(NQC):
                fill_qk(4 + 3, tcn)
            fill_qk(0, 1)
            fill_qk(1, 1)
        elif u == 3:
            fill_qk(2, 1)
            fill_qk(3, 1)
        elif u in (6, 10):
            # new qc two units ahead: q chunks for prs 0-1 here, 2-3 next
            # phase — spreading the queueing thins out the filler-dry
            # phases so no 3.4us HAM idle window trips
            fill_qk(0, units[u + 2][0])
            fill_qk(1, units[u + 2][0])
        elif u in (7, 11):
            fill_qk(2, units[u + 1][0])
            fill_qk(3, units[u + 1][0])
        elif u == 12 and r + 1 < reps:
            # next rep's first tiles, spread over phases 12-14 so no phase
            # runs filler-dry; k4 chunk writes wait only this phase's last
            # S reads of the same chunk (absorbed by the m-psum buffers)
            fill_qk(0, 0)
            fill_qk(4, 0)
            fill_qk(4, 1)
        elif u == 13 and r + 1 < reps:
            fill_qk(4, 2)
            fill_qk(4, 3)
            fill_qk(1, 0)
        elif u == 14 and r + 1 < reps:
            for tcn in range(NQC):
                fill_qk(4 + 1, tcn)
        if pr in (1, 2) and qc >= 1:
            # qc-1's zt tiles are final since end of phase (qc,0); split the
            # 8 proj chunks across two phases to spread PE filler mass
            lo = 4 * (qc - 1) + (0 if pr == 1 else 2)
            for ti in range(lo, lo + 2):
                for ocn in range(2):
                    fill_proj(ti, ocn)

        zps_e = zps_o = None
        if U >= 1:
            zps_e = pspool.tile([P, 512], F32, tag="z", bufs=2,
                                name=f"ps_ze{U-1}")
            zps_o = pspool.tile([P, 512], F32, tag="z", bufs=2,
                                name=f"ps_zo{U-1}")
        pqc, ppr = units[(U - 1) % NU] if U >= 1 else (None, None)
        a_cur = []
        for blk in range(NTT // 2):
            if U >= 1:
                # head-major: consecutive matmuls accumulate into the same
                # psum bank instead of alternating banks every matmul
                lhe, lho = 2 * ppr, 2 * ppr + 1
                for kt in (2 * blk, 2 * blk + 1):
                    nc.tensor.matmul(
                        zps_e[:], vbuf[kt][:, lhe, :],
                        a_prev[kt][:, 0, :],
                        start=(kt == 0), stop=(kt == NTT - 1),
                    )
                for kt in (2 * blk, 2 * blk + 1):
                    nc.tensor.matmul(
                        zps_o[:], vbuf[kt][:, lho, :],
                        a_prev[kt][:, 1, :],
                        start=(kt == 0), stop=(kt == NTT - 1),
                    )
            pop_fill(2 if (U == 0 or (r > 0 and u == 0)) else 1)
            for kt in (2 * blk, 2 * blk + 1):
                a_cur.append(emit_S(qc, pr, kt))
        if U >= 1:
            emit_normalize(pqc, ppr, zps_e, zps_o)
        a_prev = a_cur

    # Tail: z + normalize for the last unit, then last qc's proj.
    lqc, lpr = units[-1]
    zps_e = pspool.tile([P, 512], F32, tag="z", bufs=2, name="ps_zeL")
    zps_o = pspool.tile([P, 512], F32, tag="z", bufs=2, name="ps_zoL")
    for blk in range(NTT // 2):
        for kt in (2 * blk, 2 * blk + 1):
            emit_z_kt(zps_e, zps_o, lpr, a_prev, kt)
        pop_fill(1)
    emit_normalize(lqc, lpr, zps_e, zps_o)
    for ti in range(4 * (NQC - 1), 4 * NQC):
        for ocn in range(2):
            fill_proj(ti, ocn)
    pop_fill(len(fillers))


_NC_CACHE = None


def _get_program():
    global _NC_CACHE
    if _NC_CACHE is None:
        _NC_CACHE = build_program()
    return _NC_CACHE


def shard_inputs(x, Win_w, Win_b, proj_w, proj_b):
    """Build the 8 per-core input maps (host-side numpy)."""
    in_maps = []
    for c in range(8):
        b, g = divmod(c, 2)
        qs = slice(g * DH, (g + 1) * DH)
        ks = slice(D + g * DH, D + (g + 1) * DH)
        vs = slice(2 * D + g * DH, 2 * D + (g + 1) * DH)
        wqk = np.concatenate([Win_w[qs], Win_w[ks]], axis=0)      # [1024, 1024]
        bqk_v = np.concatenate([Win_b[qs], Win_b[ks]])            # [1024]
        in_maps.append({
            "xT": np.ascontiguousarray(x[b].T.astype(np.float16)),
            "wqkT": np.ascontiguousarray(wqk.T.astype(np.float16)),
            "wvT": np.ascontiguousarray(Win_w[vs].T.astype(np.float16)),
            "bqk": np.ascontiguousarray(
                bqk_v.reshape(NKT, P).T.astype(np.float32)),
            "bv": Win_b[vs].astype(np.float16).reshape(1, DH),
            "projT": np.ascontiguousarray(
                proj_w[:, g * DH:(g + 1) * DH].T.astype(np.float16)),
        })
    return in_maps


def combine_outputs(results, proj_b):
    out = np.empty((B, L, D), dtype=np.float32)
    pb = proj_b.astype(np.float32)
    for b in range(B):
        out[b] = results[2 * b]["out"] + results[2 * b + 1]["out"] + pb
    return out


def kernel(x, Win_w, Win_b, proj_w, proj_b):
    x = np.asarray(x, dtype=np.float32)
    Win_w = np.asarray(Win_w, dtype=np.float32)
    Win_b = np.asarray(Win_b, dtype=np.float32)
    proj_w = np.asarray(proj_w, dtype=np.float32)
    proj_b = np.asarray(proj_b, dtype=np.float32)

    nc = _get_program()
    in_maps = shard_inputs(x, Win_w, Win_b, proj_w, proj_b)
    res = run_bass_kernel_spmd(nc, in_maps, core_ids=list(range(8)))
    return combine_outputs(res.results, proj_b)
